# revision 1
# baseline (speedup 1.0000x reference)
"""Binary-weight 3x3 conv (sign(W)), NCHW, stride 1, pad 1, on 8 trn2 cores.

Full inputs:  x [32,128,56,56] f32, W [256,128,3,3] f32
Full output:  out [32,256,56,56] f32

Strategy: data-parallel over batch (4 images/core). Per core, implicit GEMM:
for each of the 9 kernel offsets, a [K=C=128, M=O=128] x [K=128, N=448]
bf16 matmul accumulating into PSUM (fp32). Images arrive host-padded with a
1-pixel zero halo so every offset is a clean shifted window.

Perf structure:
- block-outer loop: 9 accumulating matmuls per 8-row output block, then the
  PSUM bank is drained (VectorE/ScalarE alternating) while the next block's
  matmuls run. Only 4 PSUM banks needed.
- each image is loaded as four independent row-chunks (18/18/18/10 rows) and
  the weights as two 128-output halves, so the first matmul gates on ~560KB
  of DMA instead of the whole input;
- a short burst of warmup matmuls on a zeroed scratch tile runs during the
  input DMA to lift the PE out of its cold clock-gate (HAM K=4/8).

Weights are binarized (+-1, exactly representable in bf16) on host; x is cast
to bf16 on host (halves DMA traffic; rounding error ~1e-3 relative).
"""

import numpy as np
import ml_dtypes

import concourse.bacc as bacc
import concourse.mybir as mybir
from concourse.tile import TileContext
from concourse.bass_utils import run_bass_kernel_spmd

N_CORES = 8
IMGS = 4          # images per core (32 / 8)
C = 128           # input channels  = contraction dim = partitions
O = 256           # output channels
H = WD = 56
HP = WP = 58      # padded spatial
KH = KW = 3
RB = 8            # output rows per matmul block
NBLK = H // RB    # 7 blocks per image
P = 128
N_WARM = 6        # warmup matmuls

# row-chunks of the padded image; BLK_CHUNK maps block -> chunk
CHUNKS = [(0, 10), (8, 18), (24, 18), (40, 18)]  # (start_row, n_rows)
BLK_CHUNK = [0, 1, 1, 2, 2, 3, 3]

BF16 = mybir.dt.bfloat16
F32 = mybir.dt.float32


def build_nc():
    nc = bacc.Bacc(None, target_bir_lowering=False)
    x = nc.dram_tensor("x", [IMGS, C, HP, WP], BF16, kind="ExternalInput")
    wb = nc.dram_tensor("wb", [C, 2, KH, KW, P], BF16, kind="ExternalInput")
    out = nc.dram_tensor("out", [IMGS, O, H, WD], F32, kind="ExternalOutput")

    with TileContext(nc) as tc:
        with (
            tc.tile_pool(name="wpool", bufs=1) as wpool,
            tc.tile_pool(name="xpool", bufs=1) as xpool,
            tc.tile_pool(name="opool", bufs=10) as opool,
            tc.tile_pool(name="psum", bufs=6, space="PSUM") as psum_pool,
        ):
            wt = wpool.tile([P, 2, KH, KW, P], BF16, name="wt")
            wsc = wpool.tile([P, 512], BF16, name="wsc")
            nc.gpsimd.memset(wsc[:], 0.0)

            # chunk tiles: [P, IMGS, nrows, WP] per chunk index
            xts = [
                xpool.tile([P, IMGS, nr, WP], BF16, name=f"xc{ci}")
                for ci, (_, nr) in enumerate(CHUNKS)
            ]

            # DMA dispatch order = urgency order: first x chunk, then the
            # first-needed weight rows, interleaved with the rest of img0
            nc.sync.dma_start(out=xts[0][:, 0], in_=x[0, :, 0:CHUNKS[0][1]])
            nc.sync.dma_start(out=wt[:, 0, 0], in_=wb[:, 0, 0])
            nc.sync.dma_start(out=wt[:, 0, 1], in_=wb[:, 0, 1])
            nc.sync.dma_start(out=wt[:, 0, 2], in_=wb[:, 0, 2])
            nc.sync.dma_start(out=xts[1][:, 0],
                              in_=x[0, :, CHUNKS[1][0]:CHUNKS[1][0] + CHUNKS[1][1]])
            for ci, (r0, nr) in list(enumerate(CHUNKS))[2:]:
                nc.sync.dma_start(out=xts[ci][:, 0], in_=x[0, :, r0:r0 + nr])
            for kh in range(KH):
                nc.sync.dma_start(out=wt[:, 1, kh], in_=wb[:, 1, kh])
            for img in range(1, IMGS):
                for ci, (r0, nr) in enumerate(CHUNKS):
                    nc.sync.dma_start(out=xts[ci][:, img], in_=x[img, :, r0:r0 + nr])

            # warmup: PE activity during the input DMA so HAM reaches K=8/8
            warm = psum_pool.tile([P, RB, WD], F32, name="warm", tag="pst")
            for _ in range(N_WARM):
                nc.tensor.matmul(
                    warm[:], lhsT=wsc[:, :P], rhs=wsc[:, :RB * WD],
                    start=True, stop=True,
                )

            for img in range(IMGS):
                for half in range(2):
                    for blk in range(NBLK):
                        ci = BLK_CHUNK[blk]
                        cr0 = CHUNKS[ci][0]
                        pst = psum_pool.tile([P, RB, WD], F32, name="pst", tag="pst")
                        for ki in range(KH * KW):
                            kh, kw = divmod(ki, KW)
                            r0 = blk * RB + kh - cr0
                            nc.tensor.matmul(
                                pst[:],
                                lhsT=wt[:, half, kh, kw, :],
                                rhs=xts[ci][:, img, r0:r0 + RB, kw:kw + WD],
                                start=(ki == 0),
                                stop=(ki == KH * KW - 1),
                            )
                        ot = opool.tile([P, RB, WD], F32, name="ot", tag="ot")
                        if blk % 2 == 0:
                            nc.vector.tensor_copy(ot[:], pst[:])
                        else:
                            nc.scalar.copy(out=ot[:], in_=pst[:])
                        nc.sync.dma_start(
                            out=out[img, half * P:(half + 1) * P,
                                    blk * RB:(blk + 1) * RB, :],
                            in_=ot[:],
                        )
    nc.compile()
    return nc


_NC_CACHE = None


def _get_nc():
    global _NC_CACHE
    if _NC_CACHE is None:
        _NC_CACHE = build_nc()
    return _NC_CACHE


def prep_inputs(x: np.ndarray, W: np.ndarray):
    """Host-side prep: binarize weights, cast to bf16, pad, shard over cores."""
    xb = np.asarray(x).astype(ml_dtypes.bfloat16)
    xp = np.zeros((xb.shape[0], C, HP, WP), dtype=ml_dtypes.bfloat16)
    xp[:, :, 1:H + 1, 1:WD + 1] = xb
    # [O,C,3,3] -> [C, 2, KH, KW, 128]  (output-half major for split DMA)
    wsign = np.sign(np.asarray(W)).astype(ml_dtypes.bfloat16)
    wbt = np.ascontiguousarray(
        wsign.reshape(2, P, C, KH, KW).transpose(2, 0, 3, 4, 1)
    )
    xs = xp.reshape(N_CORES, IMGS, C, HP, WP)
    return [{"x": np.ascontiguousarray(xs[c]), "wb": wbt} for c in range(N_CORES)]


def kernel(x: np.ndarray, W: np.ndarray) -> np.ndarray:
    nc = _get_nc()
    in_maps = prep_inputs(x, W)
    res = run_bass_kernel_spmd(nc, in_maps, core_ids=list(range(N_CORES)))
    outs = [res.results[c]["out"] for c in range(N_CORES)]
    return np.concatenate(outs, axis=0).astype(np.float32)



# revision 4
# speedup vs baseline: 1.0568x; 1.0568x over previous
"""Binary-weight 3x3 conv (sign(W)), NCHW, stride 1, pad 1, on 8 trn2 cores.

Full inputs:  x [32,128,56,56] f32, W [256,128,3,3] f32
Full output:  out [32,256,56,56] f32

Strategy: data-parallel over batch (4 images/core). Per core, 1D Winograd
F(2,3) along H folded into an implicit GEMM: output rows are produced in
pairs from 4 transform-domain terms, cutting PE work 1.5x vs direct conv
(12 matmuls of N=392 per 14-row block vs 9 of N=448 per 8-row block).

 - input transform (DVE, bf16): per row-pair tile t:
     x~0 = xp[2t] - xp[2t+2], x~1 = xp[2t+1] + xp[2t+2],
     x~2 = xp[2t+2] - xp[2t+1], x~3 = xp[2t+1] - xp[2t+3]
 - weight transform (host, exact in bf16): per kw:
     w~ = [w0, (w0+w1+w2)/2, (w0-w1+w2)/2, w2]  with w = sign(W) in {+-1}
 - PE: m_i = sum_kw w~_i[kw]^T @ x~_i[.., kw:kw+56], accumulated in PSUM
   over kw (4 banks per block, 8 banks double-buffered)
 - inverse transform (ACT copies PSUM->SBUF bf16; DVE/Pool bf16 adds):
     out[2t]   = m0 + m1 + m2
     out[2t+1] = m1 - m2 - m3

Images arrive host-padded with a 1-pixel halo, DMAed in 4 overlapping
16-row chunks so the first matmul gates on ~2KB/partition of DMA. Warmup
matmuls on a zeroed tile ramp the PE p-state during the input DMA.
Output is written bf16 (halves DMA) and upcast on host.
"""

import numpy as np
import ml_dtypes

import concourse.bacc as bacc
import concourse.mybir as mybir
from concourse.tile import TileContext
from concourse.bass_utils import run_bass_kernel_spmd

N_CORES = 8
IMGS = 4          # images per core (32 / 8)
C = 128           # input channels = contraction dim = partitions
O = 256           # output channels
H = WD = 56
HP = WP = 58      # padded spatial
P = 128
N_WARM = 6

NCH = 4           # input row-chunks per image (16 rows each, 2-row overlap)
CHROWS = 16
RT = 7            # row-pair tiles per block
NBLK = 4          # blocks per (img, half): 4 * RT * 2 = 56 rows
NT = 28           # row-pair tiles per image

BF16 = mybir.dt.bfloat16
F32 = mybir.dt.float32


def build_nc():
    nc = bacc.Bacc(None, target_bir_lowering=False)
    x = nc.dram_tensor("x", [IMGS, C, HP, WP], BF16, kind="ExternalInput")
    wb = nc.dram_tensor("wb", [C, 2, 4, 3, P], BF16, kind="ExternalInput")
    out = nc.dram_tensor("out", [IMGS, O, H, WD], BF16, kind="ExternalOutput")

    with TileContext(nc) as tc:
        with (
            tc.tile_pool(name="wpool", bufs=1) as wpool,
            tc.tile_pool(name="xpool", bufs=1) as xpool,
            tc.tile_pool(name="tpool", bufs=1) as tpool,
            tc.tile_pool(name="cpool", bufs=2) as cpool,
            tc.tile_pool(name="opool", bufs=4) as opool,
            tc.tile_pool(name="psum", bufs=8, space="PSUM") as psum_pool,
        ):
            wt = wpool.tile([P, 2, 4, 3, P], BF16, name="wt")
            wsc = wpool.tile([P, 512], BF16, name="wsc")
            nc.gpsimd.memset(wsc[:], 0.0)

            # raw input row-chunks: chunk ch covers padded rows 14ch..14ch+15
            xc = xpool.tile([P, IMGS, NCH, CHROWS, WP], BF16, name="xc")
            # transform-domain tiles, one per Winograd term
            xw = [
                tpool.tile([P, IMGS, NT, WP], BF16, name=f"xw{i}")
                for i in range(4)
            ]

            # DMA dispatch order = urgency order
            nc.sync.dma_start(out=xc[:, 0, 0], in_=x[0, :, 0:CHROWS])
            for i in (1, 2, 0, 3):
                nc.sync.dma_start(out=wt[:, 0, i], in_=wb[:, 0, i])
            for ch in range(1, NCH):
                nc.sync.dma_start(out=xc[:, 0, ch],
                                  in_=x[0, :, 14 * ch:14 * ch + CHROWS])
            for i in (1, 2, 0, 3):
                nc.sync.dma_start(out=wt[:, 1, i], in_=wb[:, 1, i])
            for img in range(1, IMGS):
                for ch in range(NCH):
                    nc.sync.dma_start(out=xc[:, img, ch],
                                      in_=x[img, :, 14 * ch:14 * ch + CHROWS])

            # warmup: PE activity during the input DMA (p-state ramp)
            warm = psum_pool.tile([P, RT, WD], F32, name="warm", tag="pst")
            for _ in range(N_WARM):
                nc.tensor.matmul(
                    warm[:], lhsT=wsc[:, :P], rhs=wsc[:, :RT * WD],
                    start=True, stop=True,
                )

            def transform(img, ch):
                """Emit the 4 DVE input-transform ops for (img, chunk)."""
                src = xc[:, img, ch]
                t0 = RT * ch
                d0 = src[:, 0:14:2]
                d1 = src[:, 1:15:2]
                d2 = src[:, 2:16:2]
                d3 = src[:, 3:16:2]
                nc.vector.tensor_sub(xw[0][:, img, t0:t0 + RT], d0, d2)
                nc.vector.tensor_add(xw[1][:, img, t0:t0 + RT], d1, d2)
                nc.vector.tensor_sub(xw[2][:, img, t0:t0 + RT], d2, d1)
                nc.vector.tensor_sub(xw[3][:, img, t0:t0 + RT], d1, d3)

            for ch in range(NCH):
                transform(0, ch)

            for img in range(IMGS):
                for half in range(2):
                    for blk in range(NBLK):
                        # one-image transform lookahead, spread over blocks
                        if half == 0 and img + 1 < IMGS:
                            transform(img + 1, blk)

                        t0 = RT * blk
                        pst = [
                            psum_pool.tile([P, RT, WD], F32,
                                           name=f"m{i}", tag="pst")
                            for i in range(4)
                        ]
                        for i in (1, 2, 0, 3):
                            for kw in range(3):
                                nc.tensor.matmul(
                                    pst[i][:],
                                    lhsT=wt[:, half, i, kw, :],
                                    rhs=xw[i][:, img, t0:t0 + RT, kw:kw + WD],
                                    start=(kw == 0),
                                    stop=(kw == 2),
                                )

                        # inverse transform: ACT drains, DVE/Pool combine
                        c0 = cpool.tile([P, RT, WD], BF16, name="c0", tag="c0")
                        c1 = cpool.tile([P, RT, WD], BF16, name="c1", tag="c1")
                        c2 = cpool.tile([P, RT, WD], BF16, name="c2", tag="c2")
                        c3 = cpool.tile([P, RT, WD], BF16, name="c3", tag="c3")
                        s = cpool.tile([P, RT, WD], BF16, name="s", tag="s")
                        u = cpool.tile([P, RT, WD], BF16, name="u", tag="u")
                        nc.scalar.copy(out=c1[:], in_=pst[1][:])
                        nc.scalar.copy(out=c2[:], in_=pst[2][:])
                        nc.scalar.copy(out=c0[:], in_=pst[0][:])
                        nc.vector.tensor_copy(c3[:], pst[3][:])
                        nc.vector.tensor_add(s[:], c1[:], c2[:])
                        nc.vector.tensor_sub(u[:], c1[:], c2[:])

                        ot = opool.tile([P, 2 * RT, WD], BF16,
                                        name="ot", tag="ot")
                        nc.gpsimd.tensor_add(ot[:, 0:2 * RT:2], s[:], c0[:])
                        nc.gpsimd.tensor_sub(ot[:, 1:2 * RT:2], u[:], c3[:])

                        nc.sync.dma_start(
                            out=out[img, half * P:(half + 1) * P,
                                    2 * t0:2 * t0 + 2 * RT, :],
                            in_=ot[:],
                        )
    nc.compile()
    return nc


_NC_CACHE = None


def _get_nc():
    global _NC_CACHE
    if _NC_CACHE is None:
        _NC_CACHE = build_nc()
    return _NC_CACHE


def prep_inputs(x: np.ndarray, W: np.ndarray):
    """Host-side prep: binarize + Winograd-transform weights, pad x, shard."""
    xb = np.asarray(x).astype(ml_dtypes.bfloat16)
    xp = np.zeros((xb.shape[0], C, HP, WP), dtype=ml_dtypes.bfloat16)
    xp[:, :, 1:H + 1, 1:WD + 1] = xb
    # G = F(2,3) weight transform along kh; entries are exact in bf16
    G = np.array(
        [[1, 0, 0], [0.5, 0.5, 0.5], [0.5, -0.5, 0.5], [0, 0, 1]],
        dtype=np.float32,
    )
    wsign = np.sign(np.asarray(W)).astype(np.float32)  # [O,C,3,3]
    wtr = np.einsum("ih,ochw->ociw", G, wsign)         # [O,C,4,3]
    wbt = np.ascontiguousarray(
        wtr.reshape(2, P, C, 4, 3).transpose(2, 0, 3, 4, 1)
    ).astype(ml_dtypes.bfloat16)                       # [C,2,4,3,128]
    xs = xp.reshape(N_CORES, IMGS, C, HP, WP)
    return [{"x": np.ascontiguousarray(xs[c]), "wb": wbt} for c in range(N_CORES)]


def kernel(x: np.ndarray, W: np.ndarray) -> np.ndarray:
    nc = _get_nc()
    in_maps = prep_inputs(x, W)
    res = run_bass_kernel_spmd(nc, in_maps, core_ids=list(range(N_CORES)))
    outs = [res.results[c]["out"] for c in range(N_CORES)]
    return np.concatenate(outs, axis=0).astype(np.float32)
